# revision 2
# baseline (speedup 1.0000x reference)
"""Trainium2 Bass kernel for ConstOutputFilteredNormalized (segment_reduce).

y[i, j] = (x[i, j] != 0 ? f[j] : 0) / rowsum_j(masked_f[i, :]), with rows whose
masked sum is exactly 0 producing exactly 0.

Strategy: data-parallel over the batch axis — 16384 rows split into 8 shards of
2048 rows, one per NeuronCore; f (4096 floats) loaded once into partition 0 and
broadcast to all 128 SBUF partitions with a ones[1,128].T @ f[1,4096] PE matmul
(PSUM), so no 2 MiB broadcast DMA is needed. Each core processes 16 tiles of
[128 rows, 4096 cols].

The kernel is HBM-DMA-bound: every DMA transfer serializes on the shared DMA
engine pool at ~360 B/ns, so total time ~= total DMA bytes. Per tile:
  load x  [128,4096] f32 = 2 MiB  -> 5.83 us
  store y [128,4096] f16 = 1 MiB  -> 2.91 us
The output is stored as f16 (elementwise relative error ~2^-11, far inside the
2e-2 gate) and upcast to f32 on the host, halving store traffic vs f32.

Per tile compute (all hidden under the 8.7 us DMA time):
  DVE  STT: y32 = (x != 0) * f, accum_out -> den   (4.3 us)
  DVE  small: safe = den + (den == 0); recip = 1/safe
  ACT  y16 = y32 * recip (per-partition broadcast, f32->f16 cast)  (3.6 us)
Denominator: plain fp32 running-sum accumulation. Its ~1e-5 absolute error on
near-cancelling rows (min |denom| ~5e-3) contributes ~2e-3 scale-relative
output error, inside the 2e-2 gate with margin.

Loads issue from the SP sequencer and stores from the ACT sequencer (two
independent HWDGE rings) so a store waiting on its data never blocks loads.
"""

import numpy as np

B, N = 16384, 4096
NCORES = 8
ROWS_PER_CORE = B // NCORES  # 2048
P = 128
MM_N = 512  # max moving free dim per PE matmul (one PSUM bank)

_cache = {}


def _build(rows_per_core=ROWS_PER_CORE):
    import concourse.bass as bass
    import concourse.tile as tile
    from concourse import bacc, mybir

    ntiles = rows_per_core // P
    nc = bacc.Bacc(
        "TRN2",
        target_bir_lowering=False,
        debug=False,
        num_devices=NCORES,
    )
    f32 = mybir.dt.float32
    f16 = mybir.dt.float16
    x_d = nc.dram_tensor("x", [rows_per_core, N], f32, kind="ExternalInput").ap()
    f_d = nc.dram_tensor("f", [N], f32, kind="ExternalInput").ap()
    y_d = nc.dram_tensor("y", [rows_per_core, N], f16, kind="ExternalOutput").ap()

    with tile.TileContext(nc) as tc:
        with (
            tc.tile_pool(name="consts", bufs=1) as consts,
            tc.tile_pool(name="xp", bufs=4) as xp,
            tc.tile_pool(name="y32p", bufs=2) as y32p,
            tc.tile_pool(name="y16p", bufs=4) as y16p,
            tc.tile_pool(name="sp", bufs=4) as sp,
            tc.tile_pool(name="psum", bufs=1, space="PSUM") as psum,
        ):
            # f -> partition 0, then broadcast to 128 partitions via PE:
            # ones[1,128].T @ f[1,512] per PSUM bank.
            f_p0 = consts.tile([1, N], f32)
            f_row = bass.AP(
                tensor=f_d.tensor,
                offset=f_d.offset,
                ap=[[0, 1], f_d.ap[0]],
            )
            nc.sync.dma_start(out=f_p0[:], in_=f_row)
            ones = consts.tile([1, P], f32)
            nc.vector.memset(ones[:], 1.0)
            f_ps = psum.tile([P, N], f32)
            for j in range(N // MM_N):
                nc.tensor.matmul(
                    f_ps[:, j * MM_N : (j + 1) * MM_N],
                    ones[:],
                    f_p0[:, j * MM_N : (j + 1) * MM_N],
                    start=True,
                    stop=True,
                )
            f_sb = consts.tile([P, N], f32)
            nc.scalar.copy(f_sb[:], f_ps[:])

            for i in range(ntiles):
                rows = slice(i * P, (i + 1) * P)
                x_t = xp.tile([P, N], f32)
                nc.sync.dma_start(out=x_t[:], in_=x_d[rows, :])

                y32 = y32p.tile([P, N], f32)
                den = sp.tile([P, 1], f32)
                # y32 = (x != 0) * f ; den = rowsum(y32)
                nc.vector.scalar_tensor_tensor(
                    out=y32[:],
                    in0=x_t[:],
                    scalar=0.0,
                    in1=f_sb[:],
                    op0=mybir.AluOpType.not_equal,
                    op1=mybir.AluOpType.mult,
                    accum_out=den[:],
                )
                # safe = den + (den == 0); recip = 1/safe
                safe = sp.tile([P, 1], f32)
                nc.vector.tensor_scalar(
                    out=safe[:],
                    in0=den[:],
                    scalar1=0.0,
                    scalar2=None,
                    op0=mybir.AluOpType.is_equal,
                )
                nc.vector.tensor_add(out=safe[:], in0=safe[:], in1=den[:])
                nc.vector.reciprocal(out=safe[:], in_=safe[:])
                # y16 = y32 * recip (per-partition scalar broadcast + f16 cast)
                y16 = y16p.tile([P, N], f16)
                nc.scalar.mul(y16[:], y32[:], safe[:])
                nc.scalar.dma_start(out=y_d[rows, :], in_=y16[:])

    nc.compile()
    return nc


def kernel(x: np.ndarray, f: np.ndarray) -> np.ndarray:
    from concourse.bass_utils import run_bass_kernel_spmd

    if "nc" not in _cache:
        _cache["nc"] = _build()
    nc = _cache["nc"]

    x = np.ascontiguousarray(x, dtype=np.float32)
    f = np.ascontiguousarray(f, dtype=np.float32)
    assert x.shape == (B, N) and f.shape == (N,)

    shards = np.split(x, NCORES, axis=0)
    in_maps = [{"x": s, "f": f} for s in shards]
    res = run_bass_kernel_spmd(nc, in_maps, list(range(NCORES)))
    out = np.concatenate([res.results[i]["y"] for i in range(NCORES)], axis=0)
    return out.astype(np.float32)


# revision 8
# speedup vs baseline: 1.9016x; 1.9016x over previous
"""Trainium2 Bass kernel for ConstOutputFilteredNormalized (segment_reduce).

y[i, j] = (x[i, j] != 0 ? f[j] : 0) / rowsum_j(masked_f[i, :]), with rows whose
masked sum is exactly 0 producing exactly 0.

Strategy: data-parallel over the batch axis — 16384 rows split into 8 shards of
2048 rows, one per NeuronCore; f (4096 floats) loaded once into partition 0 and
broadcast to all 128 SBUF partitions with a ones[1,128].T @ f[1,4096] PE matmul
(PSUM), so no 2 MiB broadcast DMA is needed. Each core processes 16 tiles of
[128 rows, 4096 cols].

The kernel is HBM-DMA-bound: every DMA transfer serializes on the shared DMA
engine pool at ~360 B/ns, so total time ~= total DMA bytes. Per tile:
  load x  [128,4096] f32 = 2 MiB  -> 5.83 us
  store y [128,4096] f16 = 1 MiB  -> 2.91 us
The output is stored as f16 (elementwise relative error ~2^-11, far inside the
2e-2 gate) and upcast to f32 on the host, halving store traffic vs f32.

Per tile compute (all hidden under the 8.7 us DMA time):
  DVE  STT: y32 = (x != 0) * f, accum_out -> den   (4.3 us)
  DVE  small: safe = den + (den == 0); recip = 1/safe
  ACT  y16 = y32 * recip (per-partition broadcast, f32->f16 cast)  (3.6 us)
Denominator: plain fp32 running-sum accumulation. Its ~1e-5 absolute error on
near-cancelling rows (min |denom| ~5e-3) contributes ~2e-3 scale-relative
output error, inside the 2e-2 gate with margin.

Loads issue from the SP sequencer and stores from the ACT sequencer (two
independent HWDGE rings) so a store waiting on its data never blocks loads.
"""

import numpy as np

B, N = 16384, 4096
NCORES = 8
ROWS_PER_CORE = B // NCORES  # 2048
P = 128
MM_N = 512  # max moving free dim per PE matmul (one PSUM bank)

_cache = {}


def _build(rows_per_core=ROWS_PER_CORE):
    import concourse.bass as bass
    import concourse.tile as tile
    from concourse import bacc, mybir

    ntiles = rows_per_core // P
    nc = bacc.Bacc(
        "TRN2",
        target_bir_lowering=False,
        debug=False,
        num_devices=NCORES,
    )
    f32 = mybir.dt.float32
    f16 = mybir.dt.float16
    x_d = nc.dram_tensor("x", [rows_per_core, N], f32, kind="ExternalInput").ap()
    f_d = nc.dram_tensor("f", [N], f32, kind="ExternalInput").ap()
    y_d = nc.dram_tensor("y", [rows_per_core, N], f16, kind="ExternalOutput").ap()

    with tile.TileContext(nc) as tc:
        with (
            tc.tile_pool(name="consts", bufs=1) as consts,
            tc.tile_pool(name="xp", bufs=4) as xp,
            tc.tile_pool(name="y32p", bufs=2) as y32p,
            tc.tile_pool(name="y16p", bufs=4) as y16p,
            tc.tile_pool(name="sp", bufs=4) as sp,
            tc.tile_pool(name="psum", bufs=1, space="PSUM") as psum,
        ):
            # f -> partition 0, then broadcast to 128 partitions via PE:
            # ones[1,128].T @ f[1,512] per PSUM bank. The f load must be
            # FIRST on the SP queue: its 45ns transfer delays the first x
            # load trivially, while loading it any later delays f_sb by a
            # full x-tile transfer and stalls the x-buffer recycle chain
            # (measured +11.6us).
            f_p0 = consts.tile([1, N], f32)
            f_row = bass.AP(
                tensor=f_d.tensor,
                offset=f_d.offset,
                ap=[[0, 1], f_d.ap[0]],
            )
            nc.sync.dma_start(out=f_p0[:], in_=f_row)
            ones = consts.tile([1, P], f32)
            nc.vector.memset(ones[:], 1.0)
            f_ps = psum.tile([P, N], f32)
            for j in range(N // MM_N):
                nc.tensor.matmul(
                    f_ps[:, j * MM_N : (j + 1) * MM_N],
                    ones[:],
                    f_p0[:, j * MM_N : (j + 1) * MM_N],
                    start=True,
                    stop=True,
                )
            f_sb = consts.tile([P, N], f32)
            nc.scalar.copy(f_sb[:], f_ps[:])

            for i in range(ntiles):
                rows = slice(i * P, (i + 1) * P)
                last = i == ntiles - 1
                # The last tile is split column-wise so its load, STT, mul and
                # store pipeline against each other, shortening the drain tail
                # after the final x load (the DMA device is idle by then).
                nsplit = 4 if last else 1
                C = N // nsplit
                x_t = xp.tile([P, N], f32)
                for s in range(nsplit):
                    nc.sync.dma_start(
                        out=x_t[:, s * C : (s + 1) * C],
                        in_=x_d[rows, s * C : (s + 1) * C],
                    )

                y32 = y32p.tile([P, N], f32)
                den = sp.tile([P, nsplit], f32)
                # y32 = (x != 0) * f ; den = rowsum(y32)
                for s in range(nsplit):
                    cols = slice(s * C, (s + 1) * C)
                    nc.vector.scalar_tensor_tensor(
                        out=y32[:, cols],
                        in0=x_t[:, cols],
                        scalar=0.0,
                        in1=f_sb[:, cols],
                        op0=mybir.AluOpType.not_equal,
                        op1=mybir.AluOpType.mult,
                        accum_out=den[:, s : s + 1],
                    )
                if nsplit > 1:
                    dsum = sp.tile([P, 1], f32)
                    nc.vector.tensor_reduce(
                        out=dsum[:],
                        in_=den[:],
                        axis=mybir.AxisListType.X,
                        op=mybir.AluOpType.add,
                    )
                    den = dsum
                # safe = den + (den == 0); recip = 1/safe
                safe = sp.tile([P, 1], f32)
                nc.vector.scalar_tensor_tensor(
                    out=safe[:],
                    in0=den[:],
                    scalar=0.0,
                    in1=den[:],
                    op0=mybir.AluOpType.is_equal,
                    op1=mybir.AluOpType.add,
                )
                nc.vector.reciprocal(out=safe[:], in_=safe[:])
                # y16 = y32 * recip (per-partition scalar broadcast + f16 cast)
                y16 = y16p.tile([P, N], f16)
                for s in range(nsplit):
                    cols = slice(s * C, (s + 1) * C)
                    nc.scalar.mul(y16[:, cols], y32[:, cols], safe[:])
                    nc.scalar.dma_start(
                        out=y_d[rows, s * C : (s + 1) * C], in_=y16[:, cols]
                    )

    nc.compile()
    return nc


def kernel(x: np.ndarray, f: np.ndarray) -> np.ndarray:
    from concourse.bass_utils import run_bass_kernel_spmd

    if "nc" not in _cache:
        _cache["nc"] = _build()
    nc = _cache["nc"]

    x = np.ascontiguousarray(x, dtype=np.float32)
    f = np.ascontiguousarray(f, dtype=np.float32)
    assert x.shape == (B, N) and f.shape == (N,)

    shards = np.split(x, NCORES, axis=0)
    in_maps = [{"x": s, "f": f} for s in shards]
    res = run_bass_kernel_spmd(nc, in_maps, list(range(NCORES)))
    out = np.concatenate([res.results[i]["y"] for i in range(NCORES)], axis=0)
    return out.astype(np.float32)


# revision 10
# speedup vs baseline: 1.9048x; 1.0016x over previous
"""Trainium2 Bass kernel for ConstOutputFilteredNormalized (segment_reduce).

y[i, j] = (x[i, j] != 0 ? f[j] : 0) / rowsum_j(masked_f[i, :]), with rows whose
masked sum is exactly 0 producing exactly 0.

Strategy: data-parallel over the batch axis — 16384 rows split into 8 shards of
2048 rows, one per NeuronCore; f (4096 floats) loaded once into partition 0 and
broadcast to all 128 SBUF partitions with a ones[1,128].T @ f[1,4096] PE matmul
(PSUM), so no 2 MiB broadcast DMA is needed. Each core processes 16 tiles of
[128 rows, 4096 cols].

The kernel is HBM-DMA-bound: every DMA transfer serializes on the shared DMA
engine pool at ~360 B/ns, so total time ~= total DMA bytes. Per tile:
  load x  [128,4096] f32 = 2 MiB  -> 5.83 us
  store y [128,4096] f16 = 1 MiB  -> 2.91 us
The output is stored as f16 (elementwise relative error ~2^-11, far inside the
2e-2 gate) and upcast to f32 on the host, halving store traffic vs f32.
Total per core: 32 MiB loads + 16 MiB stores = 139.9 us of transfer; the
schedule reaches 143.9 us modeled (97% DMA-busy), vs 210.9 us for the f32
double-pass baseline.

Per tile compute (all hidden under the 8.7 us DMA time):
  DVE  STT: y32 = (x != 0) * f, accum_out -> den   (4.3 us)
  DVE  small: safe = den + (den == 0) fused STT; recip = 1/safe
  ACT  y16 = y32 * recip (per-partition broadcast, f32->f16 cast)  (3.4 us)
Denominator: plain fp32 running-sum accumulation. Its ~1e-5 absolute error on
near-cancelling rows (min |denom| ~5e-3) contributes ~2e-3 scale-relative
output error, inside the 2e-2 gate with margin (measured 3.5e-3 total).

Loads issue from the SP sequencer and stores from the ACT sequencer (two
independent HWDGE rings) so a store waiting on its data never blocks loads.
The f load must stay FIRST on the SP queue: its 45ns transfer barely delays
the first x load, while any later placement delays the f-broadcast chain by a
full x-tile transfer, which stalls the x-buffer recycle (measured +11.6us).
The last tile is split column-wise into quarters so its store drains sooner.
"""

import numpy as np

B, N = 16384, 4096
NCORES = 8
ROWS_PER_CORE = B // NCORES  # 2048
P = 128
MM_N = 512  # max moving free dim per PE matmul (one PSUM bank)

_cache = {}


def _build(rows_per_core=ROWS_PER_CORE):
    import concourse.bass as bass
    import concourse.tile as tile
    from concourse import bacc, mybir

    ntiles = rows_per_core // P
    nc = bacc.Bacc(
        "TRN2",
        target_bir_lowering=False,
        debug=False,
        num_devices=NCORES,
    )
    f32 = mybir.dt.float32
    f16 = mybir.dt.float16
    x_d = nc.dram_tensor("x", [rows_per_core, N], f32, kind="ExternalInput").ap()
    f_d = nc.dram_tensor("f", [N], f32, kind="ExternalInput").ap()
    y_d = nc.dram_tensor("y", [rows_per_core, N], f16, kind="ExternalOutput").ap()

    with tile.TileContext(nc) as tc:
        with (
            tc.tile_pool(name="consts", bufs=1) as consts,
            tc.tile_pool(name="xp", bufs=4) as xp,
            tc.tile_pool(name="y32p", bufs=2) as y32p,
            tc.tile_pool(name="y16p", bufs=4) as y16p,
            tc.tile_pool(name="sp", bufs=4) as sp,
            tc.tile_pool(name="psum", bufs=1, space="PSUM") as psum,
        ):
            # f -> partition 0, then broadcast to 128 partitions via PE:
            # ones[1,128].T @ f[1,512] per PSUM bank. The f load must be
            # FIRST on the SP queue: its 45ns transfer delays the first x
            # load trivially, while loading it any later delays f_sb by a
            # full x-tile transfer and stalls the x-buffer recycle chain
            # (measured +11.6us).
            f_p0 = consts.tile([1, N], f32)
            f_row = bass.AP(
                tensor=f_d.tensor,
                offset=f_d.offset,
                ap=[[0, 1], f_d.ap[0]],
            )
            nc.sync.dma_start(out=f_p0[:], in_=f_row)
            ones = consts.tile([1, P], f32)
            nc.vector.memset(ones[:], 1.0)
            f_ps = psum.tile([P, N], f32)
            for j in range(N // MM_N):
                nc.tensor.matmul(
                    f_ps[:, j * MM_N : (j + 1) * MM_N],
                    ones[:],
                    f_p0[:, j * MM_N : (j + 1) * MM_N],
                    start=True,
                    stop=True,
                )
            # Per-tile STT reads f straight from PSUM (DVE can read PSUM);
            # skipping the PSUM->SBUF copy shortens the f-ready chain that
            # gates the first tile. PSUM is otherwise unused so it can hold
            # f for the kernel's whole lifetime.
            f_sb = f_ps

            for i in range(ntiles):
                rows = slice(i * P, (i + 1) * P)
                last = i == ntiles - 1
                # The last tile is split column-wise so its load, STT, mul and
                # store pipeline against each other, shortening the drain tail
                # after the final x load (the DMA device is idle by then).
                nsplit = 4 if last else 1
                C = N // nsplit
                x_t = xp.tile([P, N], f32)
                for s in range(nsplit):
                    nc.sync.dma_start(
                        out=x_t[:, s * C : (s + 1) * C],
                        in_=x_d[rows, s * C : (s + 1) * C],
                    )

                y32 = y32p.tile([P, N], f32)
                den = sp.tile([P, nsplit], f32)
                # y32 = (x != 0) * f ; den = rowsum(y32)
                for s in range(nsplit):
                    cols = slice(s * C, (s + 1) * C)
                    nc.vector.scalar_tensor_tensor(
                        out=y32[:, cols],
                        in0=x_t[:, cols],
                        scalar=0.0,
                        in1=f_sb[:, cols],
                        op0=mybir.AluOpType.not_equal,
                        op1=mybir.AluOpType.mult,
                        accum_out=den[:, s : s + 1],
                    )
                if nsplit > 1:
                    dsum = sp.tile([P, 1], f32)
                    nc.vector.tensor_reduce(
                        out=dsum[:],
                        in_=den[:],
                        axis=mybir.AxisListType.X,
                        op=mybir.AluOpType.add,
                    )
                    den = dsum
                # safe = den + (den == 0); recip = 1/safe
                safe = sp.tile([P, 1], f32)
                nc.vector.scalar_tensor_tensor(
                    out=safe[:],
                    in0=den[:],
                    scalar=0.0,
                    in1=den[:],
                    op0=mybir.AluOpType.is_equal,
                    op1=mybir.AluOpType.add,
                )
                nc.vector.reciprocal(out=safe[:], in_=safe[:])
                # y16 = y32 * recip (per-partition scalar broadcast + f16 cast)
                y16 = y16p.tile([P, N], f16)
                for s in range(nsplit):
                    cols = slice(s * C, (s + 1) * C)
                    nc.scalar.mul(y16[:, cols], y32[:, cols], safe[:])
                    nc.scalar.dma_start(
                        out=y_d[rows, s * C : (s + 1) * C], in_=y16[:, cols]
                    )

    nc.compile()
    return nc


def kernel(x: np.ndarray, f: np.ndarray) -> np.ndarray:
    from concourse.bass_utils import run_bass_kernel_spmd

    if "nc" not in _cache:
        _cache["nc"] = _build()
    nc = _cache["nc"]

    x = np.ascontiguousarray(x, dtype=np.float32)
    f = np.ascontiguousarray(f, dtype=np.float32)
    assert x.shape == (B, N) and f.shape == (N,)

    shards = np.split(x, NCORES, axis=0)
    in_maps = [{"x": s, "f": f} for s in shards]
    res = run_bass_kernel_spmd(nc, in_maps, list(range(NCORES)))
    out = np.concatenate([res.results[i]["y"] for i in range(NCORES)], axis=0)
    return out.astype(np.float32)


# revision 13
# speedup vs baseline: 1.9128x; 1.0042x over previous
"""Trainium2 Bass kernel for ConstOutputFilteredNormalized (segment_reduce).

y[i, j] = (x[i, j] != 0 ? f[j] : 0) / rowsum_j(masked_f[i, :]), with rows whose
masked sum is exactly 0 producing exactly 0.

Strategy: data-parallel over the batch axis — 16384 rows split into 8 shards of
2048 rows, one per NeuronCore; f (4096 floats) loaded once into partition 0 and
broadcast to all 128 SBUF partitions with a ones[1,128].T @ f[1,4096] PE matmul
(PSUM), so no 2 MiB broadcast DMA is needed. Each core processes 16 tiles of
[128 rows, 4096 cols].

The kernel is HBM-DMA-bound: every DMA transfer serializes on the shared DMA
engine pool at ~360 B/ns, so total time ~= total DMA bytes. Per tile:
  load x  [128,4096] f32 = 2 MiB  -> 5.83 us
  store y [128,4096] f16 = 1 MiB  -> 2.91 us
The output is stored as f16 (elementwise relative error ~2^-11, far inside the
2e-2 gate) and upcast to f32 on the host, halving store traffic vs f32.
Total per core: 32 MiB loads + 16 MiB stores = 139.9 us of transfer; the
schedule reaches 143.9 us modeled (97% DMA-busy), vs 210.9 us for the f32
double-pass baseline.

Per tile compute (all hidden under the 8.7 us DMA time):
  DVE  STT: y32 = (x != 0) * f, accum_out -> den   (4.3 us)
  DVE  small: safe = den + (den == 0) fused STT; recip = 1/safe
  ACT  y16 = y32 * recip (per-partition broadcast, f32->f16 cast)  (3.4 us)
Denominator: plain fp32 running-sum accumulation. Its ~1e-5 absolute error on
near-cancelling rows (min |denom| ~5e-3) contributes ~2e-3 scale-relative
output error, inside the 2e-2 gate with margin (measured 3.5e-3 total).

Loads issue from the SP sequencer and stores from the ACT sequencer (two
independent HWDGE rings) so a store waiting on its data never blocks loads.
The last tile is split column-wise into quarters so its store drains sooner.
Modeled schedule is gapless on the DMA engines: 0.6us start barrier + 1.35us
first-DMA issue latency + 139.9us transfers + 1.5us sem/end barrier.
"""

import numpy as np

B, N = 16384, 4096
NCORES = 8
ROWS_PER_CORE = B // NCORES  # 2048
P = 128
MM_N = 512  # max moving free dim per PE matmul (one PSUM bank)

_cache = {}


def _build(rows_per_core=ROWS_PER_CORE):
    import concourse.bass as bass
    import concourse.tile as tile
    from concourse import bacc, mybir

    ntiles = rows_per_core // P
    nc = bacc.Bacc(
        "TRN2",
        target_bir_lowering=False,
        debug=False,
        num_devices=NCORES,
    )
    f32 = mybir.dt.float32
    f16 = mybir.dt.float16
    x_d = nc.dram_tensor("x", [rows_per_core, N], f32, kind="ExternalInput").ap()
    f_d = nc.dram_tensor("f", [N], f32, kind="ExternalInput").ap()
    y_d = nc.dram_tensor("y", [rows_per_core, N], f16, kind="ExternalOutput").ap()

    with tile.TileContext(nc) as tc:
        with (
            tc.tile_pool(name="consts", bufs=1) as consts,
            tc.tile_pool(name="xp", bufs=6) as xp,
            tc.tile_pool(name="y32p", bufs=2) as y32p,
            tc.tile_pool(name="y16p", bufs=4) as y16p,
            tc.tile_pool(name="sp", bufs=4) as sp,
            tc.tile_pool(name="psum", bufs=1, space="PSUM") as psum,
        ):
            # f -> partition 0, then broadcast to 128 partitions via PE:
            # ones[1,128].T @ f[1,512] per PSUM bank. The f load goes on the
            # ACT queue, whose slower DGE config makes the first x load (SP
            # queue) win the shared HWDGE slot, so the DMA engines start on
            # the critical 2 MiB x stream immediately; f's 46ns transfer
            # slots in behind it. The resulting later f-ready time is
            # absorbed by the 6-deep x pool (with fewer buffers, tile 4+'s
            # load gates on STT0 which gates on f: measured +2.3us; without
            # the PSUM-direct read below it cascades to +11.6us).
            f_p0 = consts.tile([1, N], f32)
            f_row = bass.AP(
                tensor=f_d.tensor,
                offset=f_d.offset,
                ap=[[0, 1], f_d.ap[0]],
            )
            nc.scalar.dma_start(out=f_p0[:], in_=f_row)
            ones = consts.tile([1, P], f32)
            nc.vector.memset(ones[:], 1.0)
            f_ps = psum.tile([P, N], f32)
            for j in range(N // MM_N):
                nc.tensor.matmul(
                    f_ps[:, j * MM_N : (j + 1) * MM_N],
                    ones[:],
                    f_p0[:, j * MM_N : (j + 1) * MM_N],
                    start=True,
                    stop=True,
                )
            # Per-tile STT reads f straight from PSUM (DVE can read PSUM);
            # skipping the PSUM->SBUF copy shortens the f-ready chain that
            # gates the first tile. PSUM is otherwise unused so it can hold
            # f for the kernel's whole lifetime.
            f_sb = f_ps

            for i in range(ntiles):
                rows = slice(i * P, (i + 1) * P)
                last = i == ntiles - 1
                # The last tile is split column-wise so its load, STT, mul and
                # store pipeline against each other, shortening the drain tail
                # after the final x load (the DMA device is idle by then).
                nsplit = 4 if last else 1
                C = N // nsplit
                x_t = xp.tile([P, N], f32)
                for s in range(nsplit):
                    nc.sync.dma_start(
                        out=x_t[:, s * C : (s + 1) * C],
                        in_=x_d[rows, s * C : (s + 1) * C],
                    )

                y32 = y32p.tile([P, N], f32)
                den = sp.tile([P, nsplit], f32)
                # y32 = (x != 0) * f ; den = rowsum(y32)
                for s in range(nsplit):
                    cols = slice(s * C, (s + 1) * C)
                    nc.vector.scalar_tensor_tensor(
                        out=y32[:, cols],
                        in0=x_t[:, cols],
                        scalar=0.0,
                        in1=f_sb[:, cols],
                        op0=mybir.AluOpType.not_equal,
                        op1=mybir.AluOpType.mult,
                        accum_out=den[:, s : s + 1],
                    )
                if nsplit > 1:
                    dsum = sp.tile([P, 1], f32)
                    nc.vector.tensor_reduce(
                        out=dsum[:],
                        in_=den[:],
                        axis=mybir.AxisListType.X,
                        op=mybir.AluOpType.add,
                    )
                    den = dsum
                # safe = den + (den == 0); recip = 1/safe
                safe = sp.tile([P, 1], f32)
                nc.vector.scalar_tensor_tensor(
                    out=safe[:],
                    in0=den[:],
                    scalar=0.0,
                    in1=den[:],
                    op0=mybir.AluOpType.is_equal,
                    op1=mybir.AluOpType.add,
                )
                nc.vector.reciprocal(out=safe[:], in_=safe[:])
                # y16 = y32 * recip (per-partition scalar broadcast + f16 cast)
                y16 = y16p.tile([P, N], f16)
                for s in range(nsplit):
                    cols = slice(s * C, (s + 1) * C)
                    nc.scalar.mul(y16[:, cols], y32[:, cols], safe[:])
                    nc.scalar.dma_start(
                        out=y_d[rows, s * C : (s + 1) * C], in_=y16[:, cols]
                    )

    nc.compile()
    return nc


def kernel(x: np.ndarray, f: np.ndarray) -> np.ndarray:
    from concourse.bass_utils import run_bass_kernel_spmd

    if "nc" not in _cache:
        _cache["nc"] = _build()
    nc = _cache["nc"]

    x = np.ascontiguousarray(x, dtype=np.float32)
    f = np.ascontiguousarray(f, dtype=np.float32)
    assert x.shape == (B, N) and f.shape == (N,)

    shards = np.split(x, NCORES, axis=0)
    in_maps = [{"x": s, "f": f} for s in shards]
    res = run_bass_kernel_spmd(nc, in_maps, list(range(NCORES)))
    out = np.concatenate([res.results[i]["y"] for i in range(NCORES)], axis=0)
    return out.astype(np.float32)
